# revision 21
# baseline (speedup 1.0000x reference)
"""Trainium2 Bass kernel for nn_DirectionalDiagram.

out[f, i, j] = X[f, i] + Y[f, j] + x[i, j]        f in [64], i,j in [1024]
  X[f, i] = (cos(t_f) - idx[i]) * 0.5 * cos(t_f)
  Y[f, j] = (sin(t_f) - idx[j]) * 0.5 * sin(t_f)
  idx[i]  = (i - 511.5) / (1024 * sqrt(2))

Sharding: the filter axis is split across the 8 NeuronCores (8 filters per
core); x is replicated.  Per core the kernel is output-bandwidth bound
(32 MiB of f32 writes); the whole computation is a single fused DVE
scalar_tensor_tensor per [128, 1024] output tile:
    out_tile = (x_tile + X_col[f,b]) + YB[f]
where X_col is a per-partition scalar column and YB[f] is Y[f, :]
broadcast across partitions.  YB is built with one-hot selector TensorE
matmuls into PSUM + ScalarE copies to SBUF, keeping the DVE (and its
SBUF ports) dedicated to the fused main loop.  Output DMAs alternate
between the two HWDGE rings (sync / scalar) since one ring tops out
around 330 GB/s.
"""

import numpy as np

W = 1024          # image side
P = 128           # SBUF partitions
NB = W // P       # 8 row-blocks
F_TOTAL = 64
N_CORES = 8
F_LOC = F_TOTAL // N_CORES   # 8 filters per core
GH = 4            # row-blocks per output DMA (2 MiB per dma_start)
HN = 512          # matmul free-dim chunk (one PSUM bank)
# aux input column layout: idx8 | s | -0.5*s | one-hot sel
AUX_S = 1024
AUX_SM = 1025
AUX_SEL = 1026
AUXW = AUX_SEL + F_LOC * P

TRACE = False     # set by test harness to capture an NTFF profile
LAST_RESULT = None

_module_cache = {}


def _build_module():
    import concourse.bacc as bacc
    import concourse.mybir as mybir
    from concourse import tile

    fp32 = mybir.dt.float32
    AOP = mybir.AluOpType

    nc = bacc.Bacc("TRN2", target_bir_lowering=False, debug=False)
    x_d = nc.dram_tensor("x", [P, NB * W], fp32, kind="ExternalInput").ap()
    aux_d = nc.dram_tensor("aux", [F_LOC, AUXW], fp32, kind="ExternalInput").ap()
    idxcol_d = nc.dram_tensor("idxcol", [P, NB], fp32, kind="ExternalInput").ap()
    cbfull_d = nc.dram_tensor("cbfull", [P, F_LOC], fp32, kind="ExternalInput").ap()
    out_d = nc.dram_tensor("out", [F_LOC, W, W], fp32, kind="ExternalOutput").ap()

    with tile.TileContext(nc) as tc:
        with (
            tc.tile_pool(name="const", bufs=1) as cpool,
            tc.tile_pool(name="outp", bufs=6) as opool,
            tc.tile_pool(name="pyb", bufs=4, space="PSUM") as pyb,
        ):
            # ---- tiny inputs first on the sync ring: they gate the whole
            # setup chain (HWDGE small-input latency floor is ~13.5us) ----
            aux_sb = cpool.tile([F_LOC, AUXW], fp32)
            nc.sync.dma_start(out=aux_sb[:, :], in_=aux_d[:, :])
            idxcol_sb = cpool.tile([P, NB], fp32)
            nc.sync.dma_start(out=idxcol_sb[:, :], in_=idxcol_d[:, :])
            cB = cpool.tile([P, F_LOC], fp32)
            nc.sync.dma_start(out=cB[:, :], in_=cbfull_d[:, :])

            # ---- x (host-pretransposed to [128, 8*1024]) in quarters,
            # split across both HWDGE rings so the 4 MiB load finishes
            # before the output stream needs the bandwidth ----
            x_sb = cpool.tile([P, NB * W], fp32)
            QW = NB * W // 4
            for q in range(4):
                eng = nc.scalar if q % 2 else nc.sync
                eng.dma_start(
                    out=x_sb[:, q * QW : (q + 1) * QW],
                    in_=x_d[:, q * QW : (q + 1) * QW],
                )

            # ---- Y rows: y_loc[f, j] = (idx[j] - s[f]) * (-0.5*s[f]) ----
            y_loc = cpool.tile([F_LOC, W], fp32)
            nc.vector.tensor_scalar(
                y_loc[:, :],
                aux_sb[:, 0:W],
                aux_sb[:, AUX_S : AUX_S + 1],
                aux_sb[:, AUX_SM : AUX_SM + 1],
                AOP.subtract,
                AOP.mult,
            )

            # ---- X columns: xc[p, f*NB+b] = (c[f] - idx[b*128+p]) * 0.5*c[f]
            t1 = cpool.tile([P, F_LOC * NB], fp32)
            nc.vector.tensor_tensor(
                t1[:, :].rearrange("p (f b) -> p f b", b=NB),
                cB[:, :].unsqueeze(2).broadcast_to([P, F_LOC, NB]),
                idxcol_sb[:, :].unsqueeze(1).broadcast_to([P, F_LOC, NB]),
                AOP.subtract,
            )
            ch = cpool.tile([P, F_LOC], fp32)
            nc.vector.tensor_scalar_mul(ch[:, :], cB[:, :], 0.5)
            xc = cpool.tile([P, F_LOC * NB], fp32)
            nc.vector.tensor_tensor(
                xc[:, :].rearrange("p (f b) -> p f b", b=NB),
                t1[:, :].rearrange("p (f b) -> p f b", b=NB),
                ch[:, :].unsqueeze(2).broadcast_to([P, F_LOC, NB]),
                AOP.mult,
            )

            # ---- YB[f] = Y[f, :] broadcast to 128 partitions via one-hot
            # selector matmuls: sel[:, f*128:(f+1)*128].T @ y_loc ----
            yb = cpool.tile([P, F_LOC * W], fp32)
            for f in range(F_LOC):
                for hf in range(W // HN):
                    ps = pyb.tile([P, HN], fp32, tag="ybp")
                    nc.tensor.matmul(
                        ps[:, :],
                        aux_sb[:, AUX_SEL + f * P : AUX_SEL + (f + 1) * P],
                        y_loc[:, hf * HN : (hf + 1) * HN],
                        start=True, stop=True,
                    )
                    nc.scalar.copy(
                        yb[:, f * W + hf * HN : f * W + (hf + 1) * HN], ps[:, :]
                    )

            # ---- main loop: one fused op per [128, 1024] output tile.
            # f0 uses 1 MiB groups on sync so the output stream starts
            # early; later groups are spread over three DMA issue paths
            # (sync HWDGE, gpsimd SWDGE rows, and the scalar HWDGE ring
            # once its yb copies are done) to keep aggregate HBM traffic
            # at the ~358 GB/s cap for the whole run. ----
            group_plan = [
                (0, 0, 2, nc.sync),
                (0, 1, 2, nc.sync),
                (0, 2, 2, nc.gpsimd),   # warm the SWDGE path early
                (0, 3, 2, nc.gpsimd),
            ]
            for f in range(1, F_LOC):
                for h in range(NB // GH):
                    if h == 1:
                        eng = nc.sync
                    elif f <= 4:
                        eng = nc.gpsimd
                    else:
                        eng = nc.scalar
                    group_plan.append((f, h, GH, eng))
            out_r = out_d.rearrange("f (g p) j -> f p g j", p=P)
            for f, h, gh, dma_eng in group_plan:
                big = opool.tile([P, GH * W], fp32, tag="big")
                for k in range(gh):
                    b = h * gh + k
                    q = f * NB + b
                    nc.vector.scalar_tensor_tensor(
                        big[:, k * W : (k + 1) * W],
                        x_sb[:, b * W : (b + 1) * W],
                        xc[:, q : q + 1],
                        yb[:, f * W : (f + 1) * W],
                        AOP.add,
                        AOP.add,
                    )
                dma_eng.dma_start(
                    out=out_r[f, :, h * gh : (h + 1) * gh, :],
                    in_=big[:, : gh * W].rearrange("p (g j) -> p g j", j=W),
                )
    nc.compile()
    return nc


def _get_module():
    if "nc" not in _module_cache:
        _module_cache["nc"] = _build_module()
    return _module_cache["nc"]


def _host_inputs(x, filters):
    x = np.asarray(x, dtype=np.float32)
    filters = np.asarray(filters, dtype=np.float32).reshape(F_TOTAL)
    # pre-transpose x to the SBUF layout [128, 8*1024] (block b at cols b*W)
    xr = np.ascontiguousarray(
        x.reshape(NB, P, W).transpose(1, 0, 2).reshape(P, NB * W)
    )
    c = np.cos(filters)
    s = np.sin(filters)
    denom = np.float32(W) * np.sqrt(np.float32(2.0))
    idx = (np.arange(W, dtype=np.float32) - np.float32(W / 2 - 0.5)) / denom
    idxcol = np.ascontiguousarray(idx.reshape(NB, P).T)  # [128, 8]
    sel = np.kron(
        np.eye(F_LOC, dtype=np.float32), np.ones((1, P), dtype=np.float32)
    )
    in_maps = []
    for core in range(N_CORES):
        sl = slice(core * F_LOC, (core + 1) * F_LOC)
        aux = np.zeros((F_LOC, AUXW), dtype=np.float32)
        aux[:, 0:W] = idx[None, :]
        aux[:, AUX_S] = s[sl]
        aux[:, AUX_SM] = np.float32(-0.5) * s[sl]
        aux[:, AUX_SEL:] = sel
        cbfull = np.ascontiguousarray(
            np.broadcast_to(c[sl][None, :], (P, F_LOC))
        )
        in_maps.append({"x": xr, "aux": aux, "idxcol": idxcol, "cbfull": cbfull})
    return in_maps


def kernel(x, filters):
    global LAST_RESULT
    import concourse.bass_utils as bass_utils

    nc = _get_module()
    in_maps = _host_inputs(x, filters)
    res = bass_utils.run_bass_kernel_spmd(
        nc,
        in_maps,
        core_ids=list(range(N_CORES)),
        trace=TRACE,
        stitch_traces=False,
    )
    LAST_RESULT = res
    return np.concatenate([r["out"] for r in res.results], axis=0)


# revision 23
# speedup vs baseline: 1.1296x; 1.1296x over previous
"""Trainium2 Bass kernel for nn_DirectionalDiagram.

out[f, i, j] = X[f, i] + Y[f, j] + x[i, j]        f in [64], i,j in [1024]
  X[f, i] = (cos(t_f) - idx[i]) * 0.5 * cos(t_f)
  Y[f, j] = (sin(t_f) - idx[j]) * 0.5 * sin(t_f)
  idx[i]  = (i - 511.5) / (1024 * sqrt(2))

Sharding: the filter axis is split across the 8 NeuronCores (8 filters per
core); x is replicated.  Per core the kernel is output-bandwidth bound
(32 MiB of f32 writes); the whole computation is a single fused DVE
scalar_tensor_tensor per [128, 1024] output tile:
    out_tile = (x_tile + X_col[f,b]) + YB[f]
where X_col is a per-partition scalar column and YB[f] is Y[f, :]
broadcast across partitions.  YB is built with one-hot selector TensorE
matmuls into PSUM + ScalarE copies to SBUF, keeping the DVE (and its
SBUF ports) dedicated to the fused main loop.  Output DMAs alternate
between the two HWDGE rings (sync / scalar) since one ring tops out
around 330 GB/s.
"""

import numpy as np

W = 1024          # image side
P = 128           # SBUF partitions
NB = W // P       # 8 row-blocks
F_TOTAL = 64
N_CORES = 8
F_LOC = F_TOTAL // N_CORES   # 8 filters per core
GH = 4            # row-blocks per output DMA (2 MiB per dma_start)
HN = 512          # matmul free-dim chunk (one PSUM bank)
# aux input column layout: idx8 | s | -0.5*s | one-hot sel
AUX_S = 1024
AUX_SM = 1025
AUX_SEL = 1026
AUXW = AUX_SEL + F_LOC * P

TRACE = False     # set by test harness to capture an NTFF profile
LAST_RESULT = None

_module_cache = {}


def _build_module():
    import concourse.bacc as bacc
    import concourse.mybir as mybir
    from concourse import tile

    fp32 = mybir.dt.float32
    AOP = mybir.AluOpType

    nc = bacc.Bacc("TRN2", target_bir_lowering=False, debug=False)
    x_d = nc.dram_tensor("x", [P, NB * W], fp32, kind="ExternalInput").ap()
    aux_d = nc.dram_tensor("aux", [F_LOC, AUXW], fp32, kind="ExternalInput").ap()
    idxcol_d = nc.dram_tensor("idxcol", [P, NB], fp32, kind="ExternalInput").ap()
    cbfull_d = nc.dram_tensor("cbfull", [P, F_LOC], fp32, kind="ExternalInput").ap()
    out_d = nc.dram_tensor("out", [F_LOC, W, W], fp32, kind="ExternalOutput").ap()

    with tile.TileContext(nc) as tc:
        with (
            tc.tile_pool(name="const", bufs=1) as cpool,
            tc.tile_pool(name="outp", bufs=6) as opool,
            tc.tile_pool(name="pyb", bufs=4, space="PSUM") as pyb,
        ):
            # ---- tiny inputs first on the sync ring: they gate the whole
            # setup chain (HWDGE small-input latency floor is ~13.5us) ----
            aux_sb = cpool.tile([F_LOC, AUXW], fp32)
            nc.sync.dma_start(out=aux_sb[:, :], in_=aux_d[:, :])
            idxcol_sb = cpool.tile([P, NB], fp32)
            nc.sync.dma_start(out=idxcol_sb[:, :], in_=idxcol_d[:, :])
            cB = cpool.tile([P, F_LOC], fp32)
            nc.sync.dma_start(out=cB[:, :], in_=cbfull_d[:, :])

            # ---- x (host-pretransposed to [128, 8*1024]) in quarters,
            # split across both HWDGE rings so the 4 MiB load finishes
            # before the output stream needs the bandwidth ----
            x_sb = cpool.tile([P, NB * W], fp32)
            QW = NB * W // 4
            for q in range(4):
                nc.scalar.dma_start(
                    out=x_sb[:, q * QW : (q + 1) * QW],
                    in_=x_d[:, q * QW : (q + 1) * QW],
                )

            # ---- Y rows: y_loc[f, j] = (idx[j] - s[f]) * (-0.5*s[f]) ----
            y_loc = cpool.tile([F_LOC, W], fp32)
            nc.vector.tensor_scalar(
                y_loc[:, :],
                aux_sb[:, 0:W],
                aux_sb[:, AUX_S : AUX_S + 1],
                aux_sb[:, AUX_SM : AUX_SM + 1],
                AOP.subtract,
                AOP.mult,
            )

            # ---- X columns: xc[p, f*NB+b] = (c[f] - idx[b*128+p]) * 0.5*c[f]
            t1 = cpool.tile([P, F_LOC * NB], fp32)
            nc.vector.tensor_tensor(
                t1[:, :].rearrange("p (f b) -> p f b", b=NB),
                cB[:, :].unsqueeze(2).broadcast_to([P, F_LOC, NB]),
                idxcol_sb[:, :].unsqueeze(1).broadcast_to([P, F_LOC, NB]),
                AOP.subtract,
            )
            ch = cpool.tile([P, F_LOC], fp32)
            nc.vector.tensor_scalar_mul(ch[:, :], cB[:, :], 0.5)
            xc = cpool.tile([P, F_LOC * NB], fp32)
            nc.vector.tensor_tensor(
                xc[:, :].rearrange("p (f b) -> p f b", b=NB),
                t1[:, :].rearrange("p (f b) -> p f b", b=NB),
                ch[:, :].unsqueeze(2).broadcast_to([P, F_LOC, NB]),
                AOP.mult,
            )

            # ---- YB[f] = Y[f, :] broadcast to 128 partitions via one-hot
            # selector matmuls: sel[:, f*128:(f+1)*128].T @ y_loc ----
            yb = cpool.tile([P, F_LOC * W], fp32)
            for f in range(F_LOC):
                for hf in range(W // HN):
                    ps = pyb.tile([P, HN], fp32, tag="ybp")
                    nc.tensor.matmul(
                        ps[:, :],
                        aux_sb[:, AUX_SEL + f * P : AUX_SEL + (f + 1) * P],
                        y_loc[:, hf * HN : (hf + 1) * HN],
                        start=True, stop=True,
                    )
                    nc.scalar.copy(
                        yb[:, f * W + hf * HN : f * W + (hf + 1) * HN], ps[:, :]
                    )

            # ---- main loop: one fused op per [128, 1024] output tile.
            # f0 uses 1 MiB groups on sync so the output stream starts
            # early; later groups are spread over three DMA issue paths
            # (sync HWDGE, gpsimd SWDGE rows, and the scalar HWDGE ring
            # once its yb copies are done) to keep aggregate HBM traffic
            # at the ~358 GB/s cap for the whole run. ----
            # Byte-balance the three DMA issue paths (they share the 16 SDMA
            # engines round-robin, so the busiest path sets the finish time):
            # sync ~12.2 MiB of outs, gpsimd ~12 MiB, scalar x (4.2) + 8 MiB.
            ring = {
                (1, 0): nc.gpsimd, (2, 0): nc.gpsimd, (3, 0): nc.gpsimd,
                (4, 0): nc.gpsimd, (5, 0): nc.gpsimd, (6, 0): nc.gpsimd,
                (1, 1): nc.sync, (3, 1): nc.sync, (5, 1): nc.sync,
                (7, 1): nc.sync,
                (2, 1): nc.scalar, (4, 1): nc.scalar, (6, 1): nc.scalar,
                (7, 0): nc.scalar,
            }
            group_plan = [(0, h, 2, nc.sync) for h in range(4)]
            for f in range(1, F_LOC):
                for h in range(NB // GH):
                    group_plan.append((f, h, GH, ring[(f, h)]))
            out_r = out_d.rearrange("f (g p) j -> f p g j", p=P)
            for f, h, gh, dma_eng in group_plan:
                big = opool.tile([P, GH * W], fp32, tag="big")
                for k in range(gh):
                    b = h * gh + k
                    q = f * NB + b
                    nc.vector.scalar_tensor_tensor(
                        big[:, k * W : (k + 1) * W],
                        x_sb[:, b * W : (b + 1) * W],
                        xc[:, q : q + 1],
                        yb[:, f * W : (f + 1) * W],
                        AOP.add,
                        AOP.add,
                    )
                dma_eng.dma_start(
                    out=out_r[f, :, h * gh : (h + 1) * gh, :],
                    in_=big[:, : gh * W].rearrange("p (g j) -> p g j", j=W),
                )
    nc.compile()
    return nc


def _get_module():
    if "nc" not in _module_cache:
        _module_cache["nc"] = _build_module()
    return _module_cache["nc"]


def _host_inputs(x, filters):
    x = np.asarray(x, dtype=np.float32)
    filters = np.asarray(filters, dtype=np.float32).reshape(F_TOTAL)
    # pre-transpose x to the SBUF layout [128, 8*1024] (block b at cols b*W)
    xr = np.ascontiguousarray(
        x.reshape(NB, P, W).transpose(1, 0, 2).reshape(P, NB * W)
    )
    c = np.cos(filters)
    s = np.sin(filters)
    denom = np.float32(W) * np.sqrt(np.float32(2.0))
    idx = (np.arange(W, dtype=np.float32) - np.float32(W / 2 - 0.5)) / denom
    idxcol = np.ascontiguousarray(idx.reshape(NB, P).T)  # [128, 8]
    sel = np.kron(
        np.eye(F_LOC, dtype=np.float32), np.ones((1, P), dtype=np.float32)
    )
    in_maps = []
    for core in range(N_CORES):
        sl = slice(core * F_LOC, (core + 1) * F_LOC)
        aux = np.zeros((F_LOC, AUXW), dtype=np.float32)
        aux[:, 0:W] = idx[None, :]
        aux[:, AUX_S] = s[sl]
        aux[:, AUX_SM] = np.float32(-0.5) * s[sl]
        aux[:, AUX_SEL:] = sel
        cbfull = np.ascontiguousarray(
            np.broadcast_to(c[sl][None, :], (P, F_LOC))
        )
        in_maps.append({"x": xr, "aux": aux, "idxcol": idxcol, "cbfull": cbfull})
    return in_maps


def kernel(x, filters):
    global LAST_RESULT
    import concourse.bass_utils as bass_utils

    nc = _get_module()
    in_maps = _host_inputs(x, filters)
    res = bass_utils.run_bass_kernel_spmd(
        nc,
        in_maps,
        core_ids=list(range(N_CORES)),
        trace=TRACE,
        stitch_traces=False,
    )
    LAST_RESULT = res
    return np.concatenate([r["out"] for r in res.results], axis=0)
